# revision 29
# baseline (speedup 1.0000x reference)
"""Rational-quadratic spline (neural spline flow) forward kernel for TRN2.

Architecture (v2 — "knots on partitions" one-hot/step matmul):

  - Data-parallel over 8 NeuronCores, batch rows sharded (62720 rows/core).
  - Per chunk of 14 row-blocks (1792 rows x 16 vars = 28672 elements):
      1. DMA x in element-major [128, (block, var)].
      2. PE transposes row-block PAIRS [128, 32] -> XT PSUM [32, pair*128]
         (f32r, exact), one DVE/ACT copy escapes XT to SBUF.
      3. PE "replication" matmuls R_gp^T @ XTS -> XB [120, cols] per
         (4-var group g, block parity): partition r = 4k+c holds x of var
         4g+c replicated over the 30 knot rows k.
      4. One compare op per (g, parity) produces ALL 29 step masks at once
         (DVE is_ge -> {0,1}, or ACT Sign -> {-1,1}); knot row k=29 has
         threshold -1e30 == always-on and carries the stream base.
      5. One small transpose-matmul per (block, group): M^T @ T_g -> E
         [elements, 4 vars x 6 streams] in PSUM: all six telescoped
         stream sums {cw, ch, AM, BM, AD, BD} per element in one shot.
      6. Rational-quadratic formula element-major across DVE/Pool/ACT;
         division via exp(ln M - ln D); outside [-5,5] select(x)/select(0).
"""

import numpy as np

TAIL_BOUND = 5.0
MIN_BIN_WIDTH = 1e-3
MIN_BIN_HEIGHT = 1e-3
MIN_DERIVATIVE = 1e-3
K = 30
V = 16
NCORES = 8

_LANES = 128
CH = 16                 # row-blocks per chunk
ROWS_CHUNK = CH * 128   # 2048
PACK = 128              # E columns per block (4 groups x 24, padded to a
                        # quarter PSUM bank so no matmul straddles a bank)
NGRP = 4                # 4-variable groups
GW = 120                # partitions per XB/mask tile (30 knots x 4 vars)
# mask producer per (g, parity, half): index = (g*2+par)*2+h; True -> DVE is_ge
_MASK_ON_DVE = [True, False, False, True, False, True, False, False,
                True, False, True, False, False, True, False, True]


# --------------------------------------------------------------------------- #
# Custom DVE ops
# --------------------------------------------------------------------------- #
_OPS_REGISTERED = {}


def _register_custom_ops():
    if _OPS_REGISTERED:
        return _OPS_REGISTERED
    import concourse.dve_ops as dve_ops
    from concourse.dve_ops import DveOp, has_src1
    from concourse.dve_spec import Spec, Src0, Src1, C0, C1, Zero, select, maxx, lower
    from concourse.dve_uop import DveOpSpec

    def mk(name, spec):
        sha = {}
        for ver in ("v3", "v4"):
            compiled = DveOpSpec(
                name=name, uops=lower(spec, ver=ver), rd1_en=has_src1(spec)
            )
            sha[ver] = compiled.sha(ver)
        op = DveOp(name, spec, subdim=False, uops_sha=sha)
        dve_ops.OPS.append(op)
        dve_ops.CUSTOM_DVE_SPECS[op.name] = op.spec
        dve_ops._SUB_OPCODE_FOR_NAME[op.name] = (
            dve_ops._CUSTOM_DVE_ROW_BASE + len(dve_ops.OPS) - 1
        )
        assert dve_ops._SUB_OPCODE_FOR_NAME[op.name] < 0x20
        return op

    MULMAX = mk(
        "RQS2_MULMAX_ANT",
        Spec(
            body=maxx(Src0 * Src1, C0),
            reference=lambda in0, in1, s0, s1, imm2: np.maximum(
                in0 * in1, s0
            ).astype(np.float32),
        ),
    )
    SUBSUB = mk(
        "RQS2_SUBSUB_ANT",
        Spec(
            body=(Src0 - Src1) - Src1,
            reference=lambda in0, in1, s0, s1, imm2: (in0 - 2.0 * in1).astype(
                np.float32
            ),
        ),
    )
    SEL_X = mk(
        "RQS2_SEL_X_ANT",
        Spec(
            body=select((Src0 >= C0) & (Src0 <= C1), Src1, Src0),
            reference=lambda in0, in1, s0, s1, imm2: np.where(
                (in0 >= s0) & (in0 <= s1), in1, in0
            ).astype(np.float32),
        ),
    )
    SEL_0 = mk(
        "RQS2_SEL_0_ANT",
        Spec(
            body=select((Src0 >= C0) & (Src0 <= C1), Src1, Zero),
            reference=lambda in0, in1, s0, s1, imm2: np.where(
                (in0 >= s0) & (in0 <= s1), in1, 0.0
            ).astype(np.float32),
        ),
    )
    _OPS_REGISTERED.update(MULMAX=MULMAX, SUBSUB=SUBSUB, SEL_X=SEL_X, SEL_0=SEL_0)
    return _OPS_REGISTERED


# --------------------------------------------------------------------------- #
# Host-side table construction
# --------------------------------------------------------------------------- #
def _softmax(x, axis=-1):
    x = x - x.max(axis=axis, keepdims=True)
    e = np.exp(x)
    return e / e.sum(axis=axis, keepdims=True)


def _softplus(x):
    return np.log1p(np.exp(-np.abs(x))) + np.maximum(x, 0)


def _knots(unnorm, min_bin, lo, hi):
    w = _softmax(unnorm.astype(np.float64), axis=-1)
    w = min_bin + (1.0 - min_bin * K) * w
    cw = np.cumsum(w, axis=-1)
    cw = np.pad(cw, ((0, 0), (1, 0)))
    cw = (hi - lo) * cw + lo
    cw[..., 0] = lo
    cw[..., -1] = hi
    return cw  # (V, K+1)


def _build_tables(uw, uh, ud):
    """Returns THR (120,4), NTHR (120,4), R (64, 8*120), T (120, 16*24)."""
    lo, hi = -TAIL_BOUND, TAIL_BOUND
    const = np.log(np.exp(1.0 - MIN_DERIVATIVE) - 1.0)
    udp = np.concatenate(
        [np.full((V, 1), const), ud.astype(np.float64), np.full((V, 1), const)],
        axis=-1,
    )
    d = MIN_DERIVATIVE + _softplus(udp)  # (V,K+1)

    cw = _knots(uw, MIN_BIN_WIDTH, lo, hi)
    chts = _knots(uh, MIN_BIN_HEIGHT, lo, hi)

    w = cw[:, 1:] - cw[:, :-1]
    h = chts[:, 1:] - chts[:, :-1]
    delta = h / w
    a = 1.0 / w
    dk = d[:, :-1]
    dk1 = d[:, 1:]

    AM = h * a * a * (1.0 - dk / delta)
    BM = h * a * dk / delta
    gam = (dk + dk1 - 2.0 * delta) / delta
    AD = -gam * a * a
    BD = gam * a

    streams = [cw[:, :-1], chts[:, :-1], AM, BM, AD, BD]  # each (V,K)
    thr = cw[:, 1:K]  # (V,29) interior knots

    THR = np.full((GW, NGRP), -1e30, dtype=np.float32)
    T = np.zeros((GW, 16 * 24), dtype=np.float32)
    Rm = np.zeros((64, 8 * GW), dtype=np.float32)
    for g in range(NGRP):
        for par in range(2):
            gp = g * 2 + par
            for c in range(4):
                v = 4 * g + c
                Rm[16 * par + v, gp * GW + np.arange(30) * 4 + c] = 1.0
                Rm[32 + 16 * par + v, gp * GW + np.arange(30) * 4 + c] = 1.0
                if par == 0:
                    for k in range(29):
                        THR[4 * k + c, g] = thr[v, k]
            for h in range(2):
                gph = gp * 2 + h
                on_dve = _MASK_ON_DVE[gph]
                for c in range(4):
                    v = 4 * g + c
                    for si, S in enumerate(streams):
                        dS = np.diff(S[v])  # (29,)
                        base = S[v, 0]
                        col = gph * 24 + c * 6 + si
                        if on_dve:
                            T[4 * np.arange(29) + c, col] = dS.astype(np.float32)
                            T[4 * 29 + c, col] = np.float32(base)
                        else:
                            T[4 * np.arange(29) + c, col] = (dS / 2.0).astype(
                                np.float32
                            )
                            T[4 * 29 + c, col] = np.float32(
                                base + dS.sum() / 2.0
                            )
    NTHR = (-THR).astype(np.float32)
    return THR, NTHR, Rm, T


# --------------------------------------------------------------------------- #
# Bass program
# --------------------------------------------------------------------------- #
_PROGRAM_CACHE = {}


def _build_program(rows_per_core):
    key = rows_per_core
    if key in _PROGRAM_CACHE:
        return _PROGRAM_CACHE[key]

    import concourse.bass as bass
    import concourse.bacc as bacc
    import concourse.tile as tile
    from concourse import mybir
    from contextlib import ExitStack

    ops = _register_custom_ops()
    MULMAX, SUBSUB = ops["MULMAX"], ops["SUBSUB"]
    SEL_X, SEL_0 = ops["SEL_X"], ops["SEL_0"]

    f32 = mybir.dt.float32
    f32r = mybir.dt.float32r
    bf16 = mybir.dt.bfloat16
    ALU = mybir.AluOpType
    AF = mybir.ActivationFunctionType

    assert rows_per_core % ROWS_CHUNK == 0
    nchunks = rows_per_core // ROWS_CHUNK
    FE = CH * 16          # element-major free size per chunk (224)
    NPAIR = CH // 2       # 7
    XTW = NPAIR * 128     # 896 cols per parity

    nc = bacc.Bacc(
        "TRN2", target_bir_lowering=False, debug=False, num_devices=NCORES
    )
    x_d = nc.dram_tensor("x", (rows_per_core, 16), f32, kind="ExternalInput")
    thr_d = nc.dram_tensor("thr", (GW, NGRP), f32, kind="ExternalInput")
    nthr_d = nc.dram_tensor("nthr", (GW, NGRP), f32, kind="ExternalInput")
    r_d = nc.dram_tensor("rmat", (64, 8 * GW), bf16, kind="ExternalInput")
    t_d = nc.dram_tensor("tbl", (GW, 16 * 24), f32, kind="ExternalInput")
    i_d = nc.dram_tensor("ident", (_LANES, _LANES), bf16, kind="ExternalInput")
    o_d = nc.dram_tensor("out", (rows_per_core, 16), f32, kind="ExternalOutput")
    l_d = nc.dram_tensor("lad", (rows_per_core, 16), f32, kind="ExternalOutput")

    x_ap, o_ap, l_ap = x_d.ap(), o_d.ap(), l_d.ap()

    with tile.TileContext(nc) as tc:
        with ExitStack() as ctx:
            cpool = ctx.enter_context(tc.tile_pool(name="const", bufs=1))
            THR = cpool.tile([GW, NGRP], f32)
            nc.sync.dma_start(THR[:], thr_d.ap())
            NTHR = cpool.tile([GW, NGRP], f32)
            nc.sync.dma_start(NTHR[:], nthr_d.ap())
            RM = cpool.tile([64, 8 * GW], bf16)
            nc.sync.dma_start(RM[:], r_d.ap())
            TT = cpool.tile([GW, 16 * 24], f32)
            nc.sync.dma_start(TT[:], t_d.ap())
            IDT = cpool.tile([_LANES, _LANES], bf16)
            nc.sync.dma_start(IDT[:], i_d.ap())

            xpool = ctx.enter_context(tc.tile_pool(name="xin", bufs=3))
            xts_pool = ctx.enter_context(tc.tile_pool(name="xts", bufs=3))
            mpool = ctx.enter_context(tc.tile_pool(name="masks", bufs=2))
            tpool = ctx.enter_context(tc.tile_pool(name="tmp", bufs=1))
            opool = ctx.enter_context(tc.tile_pool(name="outs", bufs=2))
            ps_xt = ctx.enter_context(
                tc.tile_pool(name="ps_xt", bufs=1, space="PSUM")
            )
            ps_xb = ctx.enter_context(
                tc.tile_pool(name="ps_xb", bufs=3, space="PSUM")
            )
            ps_e = ctx.enter_context(
                tc.tile_pool(name="ps_e", bufs=1, space="PSUM")
            )

            def tmp(name):
                return tpool.tile([_LANES, FE], f32, tag=name, name=name)

            def build_phase(ci):
                r0 = ci * ROWS_CHUNK
                X = xpool.tile([_LANES, FE], f32, tag="X")
                src = x_ap[r0 : r0 + ROWS_CHUNK, :].rearrange(
                    "(b p) v -> p b v", p=128
                )
                nc.sync.dma_start(
                    X[:].rearrange("p (b v) -> p b v", b=CH), src
                )

                # --- split x = hi + lo (both bf16, exact sum) -------------- #
                XHI = xpool.tile([_LANES, FE], bf16, tag="XHI")
                nc.vector.tensor_copy(XHI[:], X[:])
                XLO = xpool.tile([_LANES, FE], bf16, tag="XLO")
                nc.gpsimd.tensor_tensor(XLO[:], X[:], XHI[:], op=ALU.subtract)
                # --- transpose pairs into stacked XT [64, XTW] bf16 -------- #
                XT = ps_xt.tile([64, XTW], bf16, tag="XT")
                for pr in range(NPAIR):
                    nc.tensor.matmul(
                        XT[0:32, pr * 128 : (pr + 1) * 128],
                        XHI[:, pr * 32 : (pr + 1) * 32],
                        IDT[:],
                        is_transpose=True,
                        start=True,
                        stop=True,
                    )
                    nc.tensor.matmul(
                        XT[32:64, pr * 128 : (pr + 1) * 128],
                        XLO[:, pr * 32 : (pr + 1) * 32],
                        IDT[:],
                        is_transpose=True,
                        start=True,
                        stop=True,
                    )
                XTS = xts_pool.tile([64, XTW], bf16, tag="XTS")
                nc.scalar.copy(XTS[:], XT[:])
                return X, XTS

            def maskmm_phase(X, XTS):
                # --- per (group, parity, half): replicate, compare --------- #
                HW2 = XTW // 2
                mtiles = {}
                for g in range(NGRP):
                    for par in range(2):
                        gp = g * 2 + par
                        for h in range(2):
                            gph = gp * 2 + h
                            XB = ps_xb.tile([GW, HW2], f32, tag="XB")
                            nc.tensor.matmul(
                                XB[:],
                                RM[:, gp * GW : (gp + 1) * GW],
                                XTS[:, h * HW2 : (h + 1) * HW2],
                                start=True,
                                stop=True,
                            )
                            M = mpool.tile([GW, HW2], f32, tag=f"M{gph}")
                            if _MASK_ON_DVE[gph]:
                                nc.vector.tensor_scalar(
                                    M[:], XB[:], THR[:, g : g + 1], None,
                                    op0=ALU.is_ge,
                                )
                            else:
                                nc.scalar.activation(
                                    M[:], XB[:], AF.Sign,
                                    bias=NTHR[:, g : g + 1],
                                )
                            mtiles[(g, par, h)] = M

                # --- stream matmuls into element-major E ------------------- #
                E = ps_e.tile([_LANES, CH * PACK], f32, tag="E")
                for b in range(CH):
                    pr, par = b // 2, b % 2
                    h, prh = pr // 4, pr % 4
                    for g in range(NGRP):
                        gph = (g * 2 + par) * 2 + h
                        M = mtiles[(g, par, h)]
                        nc.tensor.matmul(
                            E[:, b * PACK + g * 24 : b * PACK + g * 24 + 24],
                            M[:, prh * 128 : (prh + 1) * 128],
                            TT[:, gph * 24 : (gph + 1) * 24],
                            start=True,
                            stop=True,
                        )
                return E

            def formula_phase(ci, X, E):
                r0 = ci * ROWS_CHUNK
                # --- formula (element-major [128, FE]) --------------------- #
                E3 = E[:].rearrange("p (b r) -> p b r", b=CH)

                def eap(s):
                    # stream-s view of E matching X's (b, v) column order
                    return E3[:, :, s : 96 : 6]

                XC = tmp("XC")
                nc.gpsimd.tensor_scalar(
                    XC[:], X[:], -TAIL_BOUND, TAIL_BOUND,
                    op0=ALU.max, op1=ALU.min,
                )
                tt = tmp("tt")
                nc.vector.tensor_tensor(tt[:], XC[:], eap(0), op=ALU.subtract)
                u = tmp("u")
                nc.vector.tensor_tensor(u[:], tt[:], eap(2), op=ALU.mult)
                t3 = tmp("t3")
                nc.vector.tensor_tensor(t3[:], u[:], eap(3), op=ALU.add)
                t4 = tmp("t4")
                nc.vector.tensor_tensor(t4[:], tt[:], eap(4), op=ALU.mult)
                t5 = tmp("t5")
                nc.vector.tensor_tensor(t5[:], t4[:], eap(5), op=ALU.add)
                q = tmp("q")
                nc.gpsimd.tensor_tensor(q[:], t5[:], tt[:], op=ALU.mult)
                D = tmp("D")
                nc.gpsimd.tensor_scalar(D[:], q[:], 1.0, None, op0=ALU.add)
                # numerator fold: out = (M + ch*D)/D, so E_ch is read early
                chD = tmp("chD")
                nc.vector.tensor_tensor(chD[:], D[:], eap(1), op=ALU.mult)
                M_ = tmp("M_")
                nc.gpsimd.tensor_tensor(M_[:], t3[:], tt[:], op=ALU.mult)
                N_ = tmp("N_")
                nc.gpsimd.tensor_tensor(N_[:], M_[:], chD[:], op=ALU.add)
                rD = tmp("rD")
                nc.vector.reciprocal(rD[:], D[:])
                outsp = tmp("outsp")
                nc.gpsimd.tensor_tensor(outsp[:], N_[:], rD[:], op=ALU.mult)
                LD = tmp("LD")
                nc.scalar.activation(LD[:], D[:], AF.Ln)
                Mp = tmp("Mp")
                nc.gpsimd.tensor_tensor(Mp[:], u[:], t3[:], op=ALU.add)
                Dp = tmp("Dp")
                nc.gpsimd.tensor_tensor(Dp[:], t4[:], t5[:], op=ALU.add)
                u1 = tmp("u1")
                nc.gpsimd.tensor_tensor(u1[:], Mp[:], D[:], op=ALU.mult)
                u2 = tmp("u2")
                nc.gpsimd.tensor_tensor(u2[:], M_[:], Dp[:], op=ALU.mult)
                P = tmp("P")
                nc.gpsimd.tensor_tensor(P[:], u1[:], u2[:], op=ALU.subtract)
                LP = tmp("LP")
                nc.scalar.activation(LP[:], P[:], AF.Ln)
                lad0 = tmp("lad0")
                nc.vector._custom_dve(
                    SUBSUB, out=lad0[:], in0=LP[:], in1=LD[:]
                )

                outs_f = opool.tile([_LANES, FE], f32, tag="outs_f")
                nc.vector._custom_dve(
                    SEL_X, out=outs_f[:], in0=X[:], in1=outsp[:],
                    s0=-TAIL_BOUND, s1=TAIL_BOUND,
                )
                lad_f = opool.tile([_LANES, FE], f32, tag="lad_f")
                nc.vector._custom_dve(
                    SEL_0, out=lad_f[:], in0=X[:], in1=lad0[:],
                    s0=-TAIL_BOUND, s1=TAIL_BOUND,
                )

                dsto = o_ap[r0 : r0 + ROWS_CHUNK, :].rearrange(
                    "(b p) v -> p b v", p=128
                )
                nc.sync.dma_start(
                    dsto, outs_f[:].rearrange("p (b v) -> p b v", b=CH)
                )
                dstl = l_ap[r0 : r0 + ROWS_CHUNK, :].rearrange(
                    "(b p) v -> p b v", p=128
                )
                nc.sync.dma_start(
                    dstl, lad_f[:].rearrange("p (b v) -> p b v", b=CH)
                )

            # two-deep software pipeline: load/transpose chunk ci+1, build
            # masks+stream sums for chunk ci, run the formula for chunk ci-1.
            loaded = None
            built = None
            for ci in range(nchunks + 2):
                nxt = build_phase(ci) if ci < nchunks else None
                if built is not None:
                    formula_phase(ci - 2, *built)
                if loaded is not None:
                    X1, XTS1 = loaded
                    E1 = maskmm_phase(X1, XTS1)
                    built = (X1, E1)
                else:
                    built = None
                loaded = nxt

    nc.compile()
    _PROGRAM_CACHE[key] = nc
    return nc


# --------------------------------------------------------------------------- #
# Entry point
# --------------------------------------------------------------------------- #
def _prepare(inputs, uw, uh, ud):
    inputs = np.asarray(inputs, dtype=np.float32)
    uw = np.asarray(uw, dtype=np.float32)
    uh = np.asarray(uh, dtype=np.float32)
    ud = np.asarray(ud, dtype=np.float32)
    B = inputs.shape[0]
    THR, NTHR, Rm, T = _build_tables(uw, uh, ud)
    import ml_dtypes
    Rm = Rm.astype(ml_dtypes.bfloat16)
    ident = np.eye(_LANES, dtype=ml_dtypes.bfloat16)

    rows_per_core = -(-B // NCORES)
    rows_per_core = ((rows_per_core + ROWS_CHUNK - 1) // ROWS_CHUNK) * ROWS_CHUNK
    Bp = rows_per_core * NCORES
    xp = np.zeros((Bp, V), dtype=np.float32)
    xp[:B] = inputs

    nc = _build_program(rows_per_core)
    in_maps = []
    for c in range(NCORES):
        xc = xp[c * rows_per_core : (c + 1) * rows_per_core]
        in_maps.append(
            {"x": xc, "thr": THR, "nthr": NTHR, "rmat": Rm, "tbl": T,
             "ident": ident}
        )
    return nc, in_maps, B, Bp, rows_per_core


def kernel(inputs, unnormalized_widths, unnormalized_heights,
           unnormalized_derivatives):
    nc, in_maps, B, Bp, rows_per_core = _prepare(
        inputs, unnormalized_widths, unnormalized_heights,
        unnormalized_derivatives,
    )
    from concourse.bass_utils import run_bass_kernel_spmd

    res = run_bass_kernel_spmd(nc, in_maps, core_ids=list(range(NCORES)))

    outs = np.empty((Bp, V), dtype=np.float32)
    lads = np.empty((Bp, V), dtype=np.float32)
    for c in range(NCORES):
        r = res.results[c]
        outs[c * rows_per_core : (c + 1) * rows_per_core] = r["out"]
        lads[c * rows_per_core : (c + 1) * rows_per_core] = r["lad"]
    return outs[:B], lads[:B]


def run_traced(inputs_dict):
    """Run once with tracing; returns HW exec time in ns (or None)."""
    nc, in_maps, B, Bp, rows_per_core = _prepare(
        inputs_dict["inputs"],
        inputs_dict["unnormalized_widths"],
        inputs_dict["unnormalized_heights"],
        inputs_dict["unnormalized_derivatives"],
    )
    from concourse.bass_utils import run_bass_kernel_spmd

    res = run_bass_kernel_spmd(
        nc, in_maps, core_ids=list(range(NCORES)), trace=True
    )
    return res.exec_time_ns


if __name__ == "__main__":
    B = 4096
    rng = np.random.default_rng(0)
    x = rng.standard_normal((B, V)).astype(np.float32)
    uw = rng.random((V, K), dtype=np.float32)
    uh = rng.random((V, K), dtype=np.float32)
    ud = rng.random((V, K - 1), dtype=np.float32)
    o, l = kernel(x, uw, uh, ud)
    print("kernel ran", o.shape, l.shape)


# revision 30
# speedup vs baseline: 1.0041x; 1.0041x over previous
"""Rational-quadratic spline (neural spline flow) forward kernel for TRN2.

Architecture (v2 — "knots on partitions" step-mask matmul):

  - Data-parallel over 8 NeuronCores, batch rows sharded (63488 rows/core).
  - Per chunk of 16 row-blocks (2048 rows x 16 vars = 32768 elements),
    two-deep software pipeline (load c+1 | masks+matmuls c | formula c-1):
      1. DMA x element-major [128, (block, var)]; split x = hi + lo (bf16
         pair, exact sum) so PE matmuls run at bf16 rate with fp32 accuracy.
      2. PE transposes row-block pairs of hi/lo -> XT PSUM [64, pair*128];
         one ACT copy escapes XT to SBUF (XTS).
      3. PE replication matmuls R^T @ XTS -> XB [120, cols] per (4-var
         group g, parity, half): partition r = 4k+c holds x (= hi + lo,
         summed by the contraction) of var 4g+c for all 30 knot rows k.
      4. ONE compare op per XB tile produces all 29 step masks at once
         (DVE is_ge -> {0,1} or ACT Sign -> {-1,1}); knot row k=29 has
         threshold -1e30 == always-on and carries the stream base.
      5. One [120]->[128,24] matmul per (block, group): M^T @ T -> E
         holds all six telescoped stream sums {cw, ch, AM, BM, AD, BD}
         per element, element-major, in one shot.
      6. Rational-quadratic formula element-major across DVE/Pool/ACT;
         out = (M + ch*D)/D via DVE reciprocal (E releases early);
         lad = ln(M'D - MD') - 2 ln(D); outside [-5,5]: select(x)/select(0).
"""

import numpy as np

TAIL_BOUND = 5.0
MIN_BIN_WIDTH = 1e-3
MIN_BIN_HEIGHT = 1e-3
MIN_DERIVATIVE = 1e-3
K = 30
V = 16
NCORES = 8

_LANES = 128
CH = 16                 # row-blocks per chunk
ROWS_CHUNK = CH * 128   # 2048
PACK = 128              # E columns per block (4 groups x 24, padded to a
                        # quarter PSUM bank so no matmul straddles a bank)
NGRP = 4                # 4-variable groups
GW = 120                # partitions per XB/mask tile (30 knots x 4 vars)
# mask producer per (g, parity, half): index = (g*2+par)*2+h; True -> DVE is_ge
_MASK_ON_DVE = [True, False, False, True, False, True, False, False,
                True, False, True, False, False, True, False, True]


# --------------------------------------------------------------------------- #
# Custom DVE ops
# --------------------------------------------------------------------------- #
_OPS_REGISTERED = {}


def _register_custom_ops():
    if _OPS_REGISTERED:
        return _OPS_REGISTERED
    import concourse.dve_ops as dve_ops
    from concourse.dve_ops import DveOp, has_src1
    from concourse.dve_spec import Spec, Src0, Src1, C0, C1, Zero, select, maxx, lower
    from concourse.dve_uop import DveOpSpec

    def mk(name, spec):
        sha = {}
        for ver in ("v3", "v4"):
            compiled = DveOpSpec(
                name=name, uops=lower(spec, ver=ver), rd1_en=has_src1(spec)
            )
            sha[ver] = compiled.sha(ver)
        op = DveOp(name, spec, subdim=False, uops_sha=sha)
        dve_ops.OPS.append(op)
        dve_ops.CUSTOM_DVE_SPECS[op.name] = op.spec
        dve_ops._SUB_OPCODE_FOR_NAME[op.name] = (
            dve_ops._CUSTOM_DVE_ROW_BASE + len(dve_ops.OPS) - 1
        )
        assert dve_ops._SUB_OPCODE_FOR_NAME[op.name] < 0x20
        return op

    MULMAX = mk(
        "RQS2_MULMAX_ANT",
        Spec(
            body=maxx(Src0 * Src1, C0),
            reference=lambda in0, in1, s0, s1, imm2: np.maximum(
                in0 * in1, s0
            ).astype(np.float32),
        ),
    )
    SUBSUB = mk(
        "RQS2_SUBSUB_ANT",
        Spec(
            body=(Src0 - Src1) - Src1,
            reference=lambda in0, in1, s0, s1, imm2: (in0 - 2.0 * in1).astype(
                np.float32
            ),
        ),
    )
    SEL_X = mk(
        "RQS2_SEL_X_ANT",
        Spec(
            body=select((Src0 >= C0) & (Src0 <= C1), Src1, Src0),
            reference=lambda in0, in1, s0, s1, imm2: np.where(
                (in0 >= s0) & (in0 <= s1), in1, in0
            ).astype(np.float32),
        ),
    )
    SEL_0 = mk(
        "RQS2_SEL_0_ANT",
        Spec(
            body=select((Src0 >= C0) & (Src0 <= C1), Src1, Zero),
            reference=lambda in0, in1, s0, s1, imm2: np.where(
                (in0 >= s0) & (in0 <= s1), in1, 0.0
            ).astype(np.float32),
        ),
    )
    _OPS_REGISTERED.update(MULMAX=MULMAX, SUBSUB=SUBSUB, SEL_X=SEL_X, SEL_0=SEL_0)
    return _OPS_REGISTERED


# --------------------------------------------------------------------------- #
# Host-side table construction
# --------------------------------------------------------------------------- #
def _softmax(x, axis=-1):
    x = x - x.max(axis=axis, keepdims=True)
    e = np.exp(x)
    return e / e.sum(axis=axis, keepdims=True)


def _softplus(x):
    return np.log1p(np.exp(-np.abs(x))) + np.maximum(x, 0)


def _knots(unnorm, min_bin, lo, hi):
    w = _softmax(unnorm.astype(np.float64), axis=-1)
    w = min_bin + (1.0 - min_bin * K) * w
    cw = np.cumsum(w, axis=-1)
    cw = np.pad(cw, ((0, 0), (1, 0)))
    cw = (hi - lo) * cw + lo
    cw[..., 0] = lo
    cw[..., -1] = hi
    return cw  # (V, K+1)


def _build_tables(uw, uh, ud):
    """Returns THR (120,4), NTHR (120,4), R (64, 8*120), T (120, 16*24)."""
    lo, hi = -TAIL_BOUND, TAIL_BOUND
    const = np.log(np.exp(1.0 - MIN_DERIVATIVE) - 1.0)
    udp = np.concatenate(
        [np.full((V, 1), const), ud.astype(np.float64), np.full((V, 1), const)],
        axis=-1,
    )
    d = MIN_DERIVATIVE + _softplus(udp)  # (V,K+1)

    cw = _knots(uw, MIN_BIN_WIDTH, lo, hi)
    chts = _knots(uh, MIN_BIN_HEIGHT, lo, hi)

    w = cw[:, 1:] - cw[:, :-1]
    h = chts[:, 1:] - chts[:, :-1]
    delta = h / w
    a = 1.0 / w
    dk = d[:, :-1]
    dk1 = d[:, 1:]

    AM = h * a * a * (1.0 - dk / delta)
    BM = h * a * dk / delta
    gam = (dk + dk1 - 2.0 * delta) / delta
    AD = -gam * a * a
    BD = gam * a

    streams = [cw[:, :-1], chts[:, :-1], AM, BM, AD, BD]  # each (V,K)
    thr = cw[:, 1:K]  # (V,29) interior knots

    THR = np.full((GW, NGRP), -1e30, dtype=np.float32)
    T = np.zeros((GW, 16 * 24), dtype=np.float32)
    Rm = np.zeros((64, 8 * GW), dtype=np.float32)
    for g in range(NGRP):
        for par in range(2):
            gp = g * 2 + par
            for c in range(4):
                v = 4 * g + c
                Rm[16 * par + v, gp * GW + np.arange(30) * 4 + c] = 1.0
                Rm[32 + 16 * par + v, gp * GW + np.arange(30) * 4 + c] = 1.0
                if par == 0:
                    for k in range(29):
                        THR[4 * k + c, g] = thr[v, k]
            for h in range(2):
                gph = gp * 2 + h
                on_dve = _MASK_ON_DVE[gph]
                for c in range(4):
                    v = 4 * g + c
                    for si, S in enumerate(streams):
                        dS = np.diff(S[v])  # (29,)
                        base = S[v, 0]
                        col = gph * 24 + c * 6 + si
                        if on_dve:
                            T[4 * np.arange(29) + c, col] = dS.astype(np.float32)
                            T[4 * 29 + c, col] = np.float32(base)
                        else:
                            T[4 * np.arange(29) + c, col] = (dS / 2.0).astype(
                                np.float32
                            )
                            T[4 * 29 + c, col] = np.float32(
                                base + dS.sum() / 2.0
                            )
    NTHR = (-THR).astype(np.float32)
    return THR, NTHR, Rm, T


# --------------------------------------------------------------------------- #
# Bass program
# --------------------------------------------------------------------------- #
_PROGRAM_CACHE = {}


def _build_program(rows_per_core):
    key = rows_per_core
    if key in _PROGRAM_CACHE:
        return _PROGRAM_CACHE[key]

    import concourse.bass as bass
    import concourse.bacc as bacc
    import concourse.tile as tile
    from concourse import mybir
    from contextlib import ExitStack

    ops = _register_custom_ops()
    MULMAX, SUBSUB = ops["MULMAX"], ops["SUBSUB"]
    SEL_X, SEL_0 = ops["SEL_X"], ops["SEL_0"]

    f32 = mybir.dt.float32
    f32r = mybir.dt.float32r
    bf16 = mybir.dt.bfloat16
    ALU = mybir.AluOpType
    AF = mybir.ActivationFunctionType

    assert rows_per_core % ROWS_CHUNK == 0
    nchunks = rows_per_core // ROWS_CHUNK
    FE = CH * 16          # element-major free size per chunk (224)
    NPAIR = CH // 2       # 7
    XTW = NPAIR * 128     # 896 cols per parity

    nc = bacc.Bacc(
        "TRN2", target_bir_lowering=False, debug=False, num_devices=NCORES
    )
    x_d = nc.dram_tensor("x", (rows_per_core, 16), f32, kind="ExternalInput")
    thr_d = nc.dram_tensor("thr", (GW, NGRP), f32, kind="ExternalInput")
    nthr_d = nc.dram_tensor("nthr", (GW, NGRP), f32, kind="ExternalInput")
    r_d = nc.dram_tensor("rmat", (64, 8 * GW), bf16, kind="ExternalInput")
    t_d = nc.dram_tensor("tbl", (GW, 16 * 24), f32, kind="ExternalInput")
    i_d = nc.dram_tensor("ident", (_LANES, _LANES), bf16, kind="ExternalInput")
    o_d = nc.dram_tensor("out", (rows_per_core, 16), f32, kind="ExternalOutput")
    l_d = nc.dram_tensor("lad", (rows_per_core, 16), f32, kind="ExternalOutput")

    x_ap, o_ap, l_ap = x_d.ap(), o_d.ap(), l_d.ap()

    with tile.TileContext(nc) as tc:
        with ExitStack() as ctx:
            cpool = ctx.enter_context(tc.tile_pool(name="const", bufs=1))
            THR = cpool.tile([GW, NGRP], f32)
            nc.sync.dma_start(THR[:], thr_d.ap())
            NTHR = cpool.tile([GW, NGRP], f32)
            nc.sync.dma_start(NTHR[:], nthr_d.ap())
            RM = cpool.tile([64, 8 * GW], bf16)
            nc.sync.dma_start(RM[:], r_d.ap())
            TT = cpool.tile([GW, 16 * 24], f32)
            nc.sync.dma_start(TT[:], t_d.ap())
            IDT = cpool.tile([_LANES, _LANES], bf16)
            nc.sync.dma_start(IDT[:], i_d.ap())

            xpool = ctx.enter_context(tc.tile_pool(name="xin", bufs=3))
            xts_pool = ctx.enter_context(tc.tile_pool(name="xts", bufs=3))
            mpool = ctx.enter_context(tc.tile_pool(name="masks", bufs=2))
            tpool = ctx.enter_context(tc.tile_pool(name="tmp", bufs=1))
            opool = ctx.enter_context(tc.tile_pool(name="outs", bufs=2))
            ps_xt = ctx.enter_context(
                tc.tile_pool(name="ps_xt", bufs=1, space="PSUM")
            )
            ps_xb = ctx.enter_context(
                tc.tile_pool(name="ps_xb", bufs=3, space="PSUM")
            )
            ps_e = ctx.enter_context(
                tc.tile_pool(name="ps_e", bufs=1, space="PSUM")
            )

            def tmp(name):
                return tpool.tile([_LANES, FE], f32, tag=name, name=name)

            def build_phase(ci):
                r0 = ci * ROWS_CHUNK
                X = xpool.tile([_LANES, FE], f32, tag="X")
                src = x_ap[r0 : r0 + ROWS_CHUNK, :].rearrange(
                    "(b p) v -> p b v", p=128
                )
                nc.sync.dma_start(
                    X[:].rearrange("p (b v) -> p b v", b=CH), src
                )

                # --- split x = hi + lo (both bf16, exact sum) -------------- #
                XHI = xpool.tile([_LANES, FE], bf16, tag="XHI")
                nc.vector.tensor_copy(XHI[:], X[:])
                XLO = xpool.tile([_LANES, FE], bf16, tag="XLO")
                nc.gpsimd.tensor_tensor(XLO[:], X[:], XHI[:], op=ALU.subtract)
                # --- transpose pairs into stacked XT [64, XTW] bf16 -------- #
                XT = ps_xt.tile([64, XTW], bf16, tag="XT")
                for pr in range(NPAIR):
                    nc.tensor.matmul(
                        XT[0:32, pr * 128 : (pr + 1) * 128],
                        XHI[:, pr * 32 : (pr + 1) * 32],
                        IDT[:],
                        is_transpose=True,
                        start=True,
                        stop=True,
                    )
                    nc.tensor.matmul(
                        XT[32:64, pr * 128 : (pr + 1) * 128],
                        XLO[:, pr * 32 : (pr + 1) * 32],
                        IDT[:],
                        is_transpose=True,
                        start=True,
                        stop=True,
                    )
                XTS = xts_pool.tile([64, XTW], bf16, tag="XTS")
                nc.scalar.copy(XTS[:], XT[:])
                return X, XTS

            def maskmm_phase(X, XTS):
                # --- per (group, parity, half): replicate, compare --------- #
                HW2 = XTW // 2
                mtiles = {}
                for g in range(NGRP):
                    for par in range(2):
                        gp = g * 2 + par
                        for h in range(2):
                            gph = gp * 2 + h
                            XB = ps_xb.tile([GW, HW2], f32, tag="XB")
                            nc.tensor.matmul(
                                XB[:],
                                RM[:, gp * GW : (gp + 1) * GW],
                                XTS[:, h * HW2 : (h + 1) * HW2],
                                start=True,
                                stop=True,
                            )
                            M = mpool.tile([GW, HW2], f32, tag=f"M{gph}")
                            if _MASK_ON_DVE[gph]:
                                nc.vector.tensor_scalar(
                                    M[:], XB[:], THR[:, g : g + 1], None,
                                    op0=ALU.is_ge,
                                )
                            else:
                                nc.scalar.activation(
                                    M[:], XB[:], AF.Sign,
                                    bias=NTHR[:, g : g + 1],
                                )
                            mtiles[(g, par, h)] = M

                # --- stream matmuls into element-major E ------------------- #
                E = ps_e.tile([_LANES, CH * PACK], f32, tag="E")
                for b in range(CH):
                    pr, par = b // 2, b % 2
                    h, prh = pr // 4, pr % 4
                    for g in range(NGRP):
                        gph = (g * 2 + par) * 2 + h
                        M = mtiles[(g, par, h)]
                        nc.tensor.matmul(
                            E[:, b * PACK + g * 24 : b * PACK + g * 24 + 24],
                            M[:, prh * 128 : (prh + 1) * 128],
                            TT[:, gph * 24 : (gph + 1) * 24],
                            start=True,
                            stop=True,
                        )
                return E

            def formula_phase(ci, X, E):
                r0 = ci * ROWS_CHUNK
                # --- formula (element-major [128, FE]) --------------------- #
                E3 = E[:].rearrange("p (b r) -> p b r", b=CH)

                def eap(s):
                    # stream-s view of E matching X's (b, v) column order
                    return E3[:, :, s : 96 : 6]

                XC = tmp("XC")
                nc.gpsimd.tensor_scalar(
                    XC[:], X[:], -TAIL_BOUND, TAIL_BOUND,
                    op0=ALU.max, op1=ALU.min,
                )
                tt = tmp("tt")
                nc.vector.tensor_tensor(tt[:], XC[:], eap(0), op=ALU.subtract)
                u = tmp("u")
                nc.vector.tensor_tensor(u[:], tt[:], eap(2), op=ALU.mult)
                t3 = tmp("t3")
                nc.vector.tensor_tensor(t3[:], u[:], eap(3), op=ALU.add)
                t4 = tmp("t4")
                nc.vector.tensor_tensor(t4[:], tt[:], eap(4), op=ALU.mult)
                t5 = tmp("t5")
                nc.vector.tensor_tensor(t5[:], t4[:], eap(5), op=ALU.add)
                q = tmp("q")
                nc.gpsimd.tensor_tensor(q[:], t5[:], tt[:], op=ALU.mult)
                D = tmp("D")
                nc.gpsimd.tensor_scalar(D[:], q[:], 1.0, None, op0=ALU.add)
                # numerator fold: out = (M + ch*D)/D, so E_ch is read early
                chD = tmp("chD")
                nc.vector.tensor_tensor(chD[:], D[:], eap(1), op=ALU.mult)
                M_ = tmp("M_")
                nc.gpsimd.tensor_tensor(M_[:], t3[:], tt[:], op=ALU.mult)
                N_ = tmp("N_")
                nc.gpsimd.tensor_tensor(N_[:], M_[:], chD[:], op=ALU.add)
                rD = tmp("rD")
                nc.vector.reciprocal(rD[:], D[:])
                outsp = tmp("outsp")
                nc.gpsimd.tensor_tensor(outsp[:], N_[:], rD[:], op=ALU.mult)
                LD = tmp("LD")
                nc.scalar.activation(LD[:], D[:], AF.Ln)
                Mp = tmp("Mp")
                nc.gpsimd.tensor_tensor(Mp[:], u[:], t3[:], op=ALU.add)
                Dp = tmp("Dp")
                nc.gpsimd.tensor_tensor(Dp[:], t4[:], t5[:], op=ALU.add)
                u1 = tmp("u1")
                nc.gpsimd.tensor_tensor(u1[:], Mp[:], D[:], op=ALU.mult)
                u2 = tmp("u2")
                nc.gpsimd.tensor_tensor(u2[:], M_[:], Dp[:], op=ALU.mult)
                P = tmp("P")
                nc.gpsimd.tensor_tensor(P[:], u1[:], u2[:], op=ALU.subtract)
                LP = tmp("LP")
                nc.scalar.activation(LP[:], P[:], AF.Ln)
                lad0 = tmp("lad0")
                nc.vector._custom_dve(
                    SUBSUB, out=lad0[:], in0=LP[:], in1=LD[:]
                )

                outs_f = opool.tile([_LANES, FE], f32, tag="outs_f")
                nc.vector._custom_dve(
                    SEL_X, out=outs_f[:], in0=X[:], in1=outsp[:],
                    s0=-TAIL_BOUND, s1=TAIL_BOUND,
                )
                lad_f = opool.tile([_LANES, FE], f32, tag="lad_f")
                nc.vector._custom_dve(
                    SEL_0, out=lad_f[:], in0=X[:], in1=lad0[:],
                    s0=-TAIL_BOUND, s1=TAIL_BOUND,
                )

                dsto = o_ap[r0 : r0 + ROWS_CHUNK, :].rearrange(
                    "(b p) v -> p b v", p=128
                )
                nc.sync.dma_start(
                    dsto, outs_f[:].rearrange("p (b v) -> p b v", b=CH)
                )
                dstl = l_ap[r0 : r0 + ROWS_CHUNK, :].rearrange(
                    "(b p) v -> p b v", p=128
                )
                nc.sync.dma_start(
                    dstl, lad_f[:].rearrange("p (b v) -> p b v", b=CH)
                )

            # two-deep software pipeline: load/transpose chunk ci+1, build
            # masks+stream sums for chunk ci, run the formula for chunk ci-1.
            loaded = None
            built = None
            for ci in range(nchunks + 2):
                nxt = build_phase(ci) if ci < nchunks else None
                if loaded is not None:
                    X1, XTS1 = loaded
                    E1 = maskmm_phase(X1, XTS1)
                    cur = (X1, E1)
                else:
                    cur = None
                if built is not None:
                    formula_phase(ci - 2, *built)
                built = cur
                loaded = nxt

    nc.compile()
    _PROGRAM_CACHE[key] = nc
    return nc


# --------------------------------------------------------------------------- #
# Entry point
# --------------------------------------------------------------------------- #
def _prepare(inputs, uw, uh, ud):
    inputs = np.asarray(inputs, dtype=np.float32)
    uw = np.asarray(uw, dtype=np.float32)
    uh = np.asarray(uh, dtype=np.float32)
    ud = np.asarray(ud, dtype=np.float32)
    B = inputs.shape[0]
    THR, NTHR, Rm, T = _build_tables(uw, uh, ud)
    import ml_dtypes
    Rm = Rm.astype(ml_dtypes.bfloat16)
    ident = np.eye(_LANES, dtype=ml_dtypes.bfloat16)

    rows_per_core = -(-B // NCORES)
    rows_per_core = ((rows_per_core + ROWS_CHUNK - 1) // ROWS_CHUNK) * ROWS_CHUNK
    Bp = rows_per_core * NCORES
    xp = np.zeros((Bp, V), dtype=np.float32)
    xp[:B] = inputs

    nc = _build_program(rows_per_core)
    in_maps = []
    for c in range(NCORES):
        xc = xp[c * rows_per_core : (c + 1) * rows_per_core]
        in_maps.append(
            {"x": xc, "thr": THR, "nthr": NTHR, "rmat": Rm, "tbl": T,
             "ident": ident}
        )
    return nc, in_maps, B, Bp, rows_per_core


def kernel(inputs, unnormalized_widths, unnormalized_heights,
           unnormalized_derivatives):
    nc, in_maps, B, Bp, rows_per_core = _prepare(
        inputs, unnormalized_widths, unnormalized_heights,
        unnormalized_derivatives,
    )
    from concourse.bass_utils import run_bass_kernel_spmd

    res = run_bass_kernel_spmd(nc, in_maps, core_ids=list(range(NCORES)))

    outs = np.empty((Bp, V), dtype=np.float32)
    lads = np.empty((Bp, V), dtype=np.float32)
    for c in range(NCORES):
        r = res.results[c]
        outs[c * rows_per_core : (c + 1) * rows_per_core] = r["out"]
        lads[c * rows_per_core : (c + 1) * rows_per_core] = r["lad"]
    return outs[:B], lads[:B]


def run_traced(inputs_dict):
    """Run once with tracing; returns HW exec time in ns (or None)."""
    nc, in_maps, B, Bp, rows_per_core = _prepare(
        inputs_dict["inputs"],
        inputs_dict["unnormalized_widths"],
        inputs_dict["unnormalized_heights"],
        inputs_dict["unnormalized_derivatives"],
    )
    from concourse.bass_utils import run_bass_kernel_spmd

    res = run_bass_kernel_spmd(
        nc, in_maps, core_ids=list(range(NCORES)), trace=True
    )
    return res.exec_time_ns


if __name__ == "__main__":
    B = 4096
    rng = np.random.default_rng(0)
    x = rng.standard_normal((B, V)).astype(np.float32)
    uw = rng.random((V, K), dtype=np.float32)
    uh = rng.random((V, K), dtype=np.float32)
    ud = rng.random((V, K - 1), dtype=np.float32)
    o, l = kernel(x, uw, uh, ud)
    print("kernel ran", o.shape, l.shape)
